# revision 6
# baseline (speedup 1.0000x reference)
"""LoRA attention Bass kernel for 8x Trainium2 NeuronCores.

Sharding (Megatron tensor-parallel over heads):
  - Each of the 8 cores owns 2 heads (128 projection columns).
  - q/k/v projections column-sharded; out projection row-sharded;
    per-core partial outputs are summed on the host.
  - LoRA is merged into the base weights on the host (w_eff = w + a@u*scaling),
    exact up to fp32 rounding; host casts x/wq/wk/wv/wo to bf16.

Fused single-loop pipeline (v2). The softmax exp on the scalar engine is
the hard floor (256 ACTIVATE calls x ~1.1us = ~287us/core); everything
else is organized to hide under that pace:
  - No separate projection phase: projection chunk c (512 seq rows of
    q/k/v for batch c//4) is emitted interleaved inside attention outer
    c-4's t-loop, so the PE fills exp-wait gaps with projection matmuls
    while ACT runs flat out. Batch 0 projects in a short prologue.
  - qT/kT computed transposed ([proj_col, seq]) at full PE rate (N=512);
    PSUM evicted by DVE with fused bias-add + bf16 cast (ACT does exp ONLY).
  - v natural layout ([t, d_k] slabs with fused ones columns for the
    softmax denominator) produced by DMA-xbar transposes of the vT
    eviction - zero PE/ACT cost.
  - Attention inner loop per (batch, s-chunk) outer: S^T = K @ Q^T with
    both heads as row-tiled concurrent matmuls (K=64 at tile_position
    (0,0)/(64,0)) into a [128,1024] 2-bank PSUM tile; one exp over
    [128,1024] on ACT into bf16; P@V with lhsT=[v | ones] so the softmax
    denominator falls out of the same matmul (row 64). PV emitted two
    t-steps late so it never heads-of-line blocks on exp.
  - Normalization (DVE/gpsimd, off critical path): denom row lane-shifted
    to partition 0 by a tiny SBUF->SBUF DMA, reciprocal_approx_fast,
    gpsimd partition_broadcast, DVE multiply; head-B half lane-shifted by
    another local DMA. Out-projection of outer i deferred into outer
    i+1's t-loop; PSUM evicted by DVE, DMAed to DRAM.

PSUM budget (8 banks): scores 2x2 + pv 2 + proj 1 + outproj 1.
"""

import numpy as np

import concourse.bass as bass
import concourse.mybir as mybir
import concourse.tile as tile
from concourse import bacc
from concourse.bass_utils import run_bass_kernel_spmd

F32 = mybir.dt.float32
BF16 = mybir.dt.bfloat16
AF = mybir.ActivationFunctionType

N_CORES = 8

# Full-problem dims (hardcoded per spec)
D_MODEL = 1024
N_HEADS = 16
D_K = 64
LORA_R = 8
SCALING = 2.0
B = 4
S = 2048


class Cfg:
    """Kernel build configuration."""

    def __init__(self, b=B, s=S, d=D_MODEL, cpc=128, dk=D_K):
        self.b = b                     # batches
        self.s = s                     # seq per batch
        self.d = d                     # model dim (contraction for projections)
        self.cpc = cpc                 # projection cols per core (2 heads x 64)
        self.dk = dk                   # head dim
        self.seq = b * s               # total rows
        self.nkc = d // 128            # k chunks for projections
        self.sc = 512                  # s-chunk width (free dim of matmuls)
        self.nsc = self.seq // self.sc  # s chunks over the whole input
        self.nt = s // 128             # t chunks per batch
        self.nsb = s // self.sc        # s chunks per batch


def _build_nc(cfg: Cfg, dump: bool = False):
    c = cfg
    nc = bacc.Bacc("TRN2", target_bir_lowering=False, debug=False,
                   num_devices=N_CORES)

    if dump:
        qT_d = nc.dram_tensor("qT_d", [128, c.seq], BF16, kind="ExternalOutput").ap()
        kT_d = nc.dram_tensor("kT_d", [128, c.seq], BF16, kind="ExternalOutput").ap()
        v_d = nc.dram_tensor("v_d", [128, (c.seq // 128) * 160], BF16, kind="ExternalOutput").ap()
        e_d = nc.dram_tensor("e_d", [128, 1024], BF16, kind="ExternalOutput").ap()
        pv_d = nc.dram_tensor("pv_d", [65, 1024], F32, kind="ExternalOutput").ap()
        x_d = nc.dram_tensor("x_d", [128, 8, 512], BF16, kind="ExternalOutput").ap()

    xT = nc.dram_tensor("xT", [c.d, c.seq], BF16, kind="ExternalInput").ap()
    wq = nc.dram_tensor("wq", [c.d, c.cpc], BF16, kind="ExternalInput").ap()
    wk = nc.dram_tensor("wk", [c.d, c.cpc], BF16, kind="ExternalInput").ap()
    wv = nc.dram_tensor("wv", [c.d, c.cpc], BF16, kind="ExternalInput").ap()
    wo = nc.dram_tensor("wo", [c.cpc, c.d], BF16, kind="ExternalInput").ap()
    bq = nc.dram_tensor("bq", [c.cpc, 1], F32, kind="ExternalInput").ap()
    bk = nc.dram_tensor("bk", [c.cpc, 1], F32, kind="ExternalInput").ap()
    out = nc.dram_tensor("out", [c.seq, c.d], F32, kind="ExternalOutput").ap()

    dk = c.dk
    nt = c.nt                     # t-steps per outer (128-row chunks per batch)
    ntc = c.seq // 128            # global 128-row t-chunk count
    n_outers = c.b * c.nsb
    n_chunks = n_outers           # projection chunks, same 512-row granularity
    tpc = c.sc // 128             # t-chunks per projection chunk (4)
    VW = 160                      # v slab width: [0:64]=A, 64=onesA, [80:144]=B, 144=onesB
    HB = 80    # 32-byte-aligned slab offset (DMA-xbar transpose corrupts at 16B-only alignment)

    # per-t-step schedules inside an outer
    oj_sched: dict[int, list[int]] = {}
    for j in range(c.sc // 128):
        oj_sched.setdefault(min(5 + 3 * j, nt - 1), []).append(j)
    pp_sched: dict[int, list[int]] = {}
    for p in range(6):
        pp_sched.setdefault(min(2 + 2 * p, nt - 1), []).append(p)

    with tile.TileContext(nc) as tc:
        with tc.tile_pool(name="persist", bufs=1) as persist:
            qT_sb = persist.tile([128, c.seq], BF16, tag="qT")
            kT_sb = persist.tile([128, c.seq], BF16, tag="kT")
            v_sb = persist.tile([128, ntc, VW], BF16, tag="v")
            wq_sb = persist.tile([128, c.nkc, c.cpc], BF16, tag="wq")
            wk_sb = persist.tile([128, c.nkc, c.cpc], BF16, tag="wk")
            wv_sb = persist.tile([128, c.nkc, c.cpc], BF16, tag="wv")
            wo_sb = persist.tile([c.cpc, c.d], BF16, tag="wo")
            bq_sb = persist.tile([c.cpc, 1], F32, tag="bq")
            bk_sb = persist.tile([c.cpc, 1], F32, tag="bk")

            nc.sync.dma_start(out=wq_sb[:], in_=wq.rearrange("(kc p) m -> p kc m", p=128))
            nc.sync.dma_start(out=wk_sb[:], in_=wk.rearrange("(kc p) m -> p kc m", p=128))
            nc.sync.dma_start(out=wv_sb[:], in_=wv.rearrange("(kc p) m -> p kc m", p=128))
            nc.sync.dma_start(out=wo_sb[:], in_=wo[:])
            nc.sync.dma_start(out=bq_sb[:], in_=bq[:])
            nc.sync.dma_start(out=bk_sb[:], in_=bk[:])

            # ones columns for the fused softmax denominator
            ones_f32 = persist.tile([128, 1], F32, tag="ones_f32")
            nc.vector.memset(ones_f32[:], 1.0)
            nc.vector.tensor_copy(
                v_sb[:, :, dk:dk + 1],
                ones_f32[:].unsqueeze(1).to_broadcast([128, ntc, 1]))
            nc.vector.tensor_copy(
                v_sb[:, :, HB + dk:HB + dk + 1],
                ones_f32[:].unsqueeze(1).to_broadcast([128, ntc, 1]))

            bq_bc = bq_sb[:].to_broadcast([c.cpc, c.sc])
            bk_bc = bk_sb[:].to_broadcast([c.cpc, c.sc])

            with tc.tile_pool(name="xin", bufs=5) as xpool, \
                 tc.tile_pool(name="vt", bufs=2) as vtpool, \
                 tc.tile_pool(name="sps", bufs=2, space="PSUM") as spool, \
                 tc.tile_pool(name="pvps", bufs=1, space="PSUM") as pvpool, \
                 tc.tile_pool(name="mps", bufs=1, space="PSUM") as mpool, \
                 tc.tile_pool(name="exp", bufs=6) as epool, \
                 tc.tile_pool(name="norm", bufs=3) as npool, \
                 tc.tile_pool(name="pvs", bufs=2) as pvspool, \
                 tc.tile_pool(name="bcs", bufs=2) as bcspool, \
                 tc.tile_pool(name="rec", bufs=3) as rpool, \
                 tc.tile_pool(name="osb", bufs=3) as osbpool:

                x_tiles: dict[int, bass.AP] = {}
                proj_ps: dict[tuple, bass.AP] = {}

                def issue_x(ci):
                    x_t = xpool.tile([128, c.nkc, c.sc], BF16, tag="x",
                                     name=f"x_{ci}")
                    nc.sync.dma_start(
                        out=x_t[:],
                        in_=xT.rearrange("(kc p) s -> p kc s", p=128)
                        [:, :, ci * c.sc:(ci + 1) * c.sc])
                    x_tiles[ci] = x_t

                def emit_proj(ci, part):
                    """part 0..5 = (q|k|v) x (kc half 0|1) for chunk ci."""
                    which, half = divmod(part, 2)
                    w_sb = (wq_sb, wk_sb, wv_sb)[which]
                    x_t = x_tiles[ci]
                    if half == 0:
                        proj_ps[(ci, which)] = mpool.tile(
                            [128, c.sc], F32, tag="pp",
                            name=f"pp_{ci}_{which}")
                    ps = proj_ps[(ci, which)]
                    h4 = c.nkc // 2
                    for kc in range(half * h4, half * h4 + h4):
                        nc.tensor.matmul(ps[:], w_sb[:, kc, :], x_t[:, kc, :],
                                         start=(kc == 0), stop=(kc == c.nkc - 1))
                    if half == 1:
                        s0c = ci * c.sc
                        if which == 0:
                            nc.vector.tensor_tensor(
                                qT_sb[:, s0c:s0c + c.sc], ps[:], bq_bc,
                                mybir.AluOpType.add)
                        elif which == 1:
                            nc.vector.tensor_tensor(
                                kT_sb[:, s0c:s0c + c.sc], ps[:], bk_bc,
                                mybir.AluOpType.add)
                        else:
                            vt = vtpool.tile([128, c.sc], BF16, tag="vt",
                                             name=f"vt_{ci}")
                            nc.vector.tensor_copy(vt[:], ps[:])
                            tc0 = ci * tpc
                            nc.sync.dma_start_transpose(
                                out=v_sb[:, tc0:tc0 + tpc, 0:dk],
                                in_=vt[0:dk, :])
                            nc.sync.dma_start_transpose(
                                out=v_sb[:, tc0:tc0 + tpc, HB:HB + dk],
                                in_=vt[dk:2 * dk, :])
                        del proj_ps[(ci, which)]

                def emit_outproj_chunk(norm128, s0, j):
                    o_t = osbpool.tile([128, c.d], F32, tag="osb",
                                       name=f"o_t_{s0}_{j}")
                    ew = 512
                    for e in range(c.d // ew):
                        o_ps = mpool.tile([128, ew], F32, tag="o",
                                          name=f"o_ps_{s0}_{j}_{e}")
                        nc.tensor.matmul(
                            o_ps[:],
                            norm128[:, j * 128:(j + 1) * 128],
                            wo_sb[:, e * ew:(e + 1) * ew],
                            start=True, stop=True)
                        nc.vector.tensor_copy(
                            o_t[:, e * ew:(e + 1) * ew], o_ps[:])
                    nc.sync.dma_start(
                        out=out[s0 + j * 128:s0 + (j + 1) * 128, :],
                        in_=o_t[:])

                # ---------------- prologue: batch 0 projections ----------------
                for ci in range(c.nsb + 1):
                    if ci < n_chunks:
                        issue_x(ci)
                for ci in range(c.nsb):
                    for part in range(6):
                        emit_proj(ci, part)

                # ---------------- fused attention + projection loop ----------------
                pending = None
                for o in range(n_outers):
                    b_i, sb_i = divmod(o, c.nsb)
                    s0 = o * c.sc
                    hc = o + c.nsb if o + c.nsb < n_chunks else None
                    if hc is not None and hc + 1 < n_chunks:
                        issue_x(hc + 1)

                    pv_ab = pvpool.tile([dk + 1, 2 * c.sc], F32, tag="pv")

                    def emit_pv(t, e_ab):
                        tci = b_i * nt + t
                        nc.tensor.matmul(
                            pv_ab[:, 0:c.sc], v_sb[:, tci, 0:dk + 1],
                            e_ab[:, 0:c.sc],
                            start=(t == 0), stop=(t == nt - 1))
                        nc.tensor.matmul(
                            pv_ab[:, c.sc:2 * c.sc],
                            v_sb[:, tci, HB:HB + dk + 1],
                            e_ab[:, c.sc:2 * c.sc],
                            start=(t == 0), stop=(t == nt - 1))

                    e_hist = []
                    for t in range(nt):
                        t0 = b_i * c.s + t * 128
                        s_ab = spool.tile([128, 2 * c.sc], F32, tag="s")
                        nc.tensor.matmul(
                            s_ab[:, 0:c.sc],
                            kT_sb[0:dk, t0:t0 + 128],
                            qT_sb[0:dk, s0:s0 + c.sc],
                            start=True, stop=True,
                            tile_position=(0, 0))
                        nc.tensor.matmul(
                            s_ab[:, c.sc:2 * c.sc],
                            kT_sb[dk:2 * dk, t0:t0 + 128],
                            qT_sb[dk:2 * dk, s0:s0 + c.sc],
                            start=True, stop=True,
                            tile_position=(64, 0))
                        e_ab = epool.tile([128, 2 * c.sc], BF16, tag="e")
                        nc.scalar.activation(e_ab[:], s_ab[:], AF.Exp,
                                             scale=1.0 / np.sqrt(dk))
                        if len(e_hist) == 2:
                            emit_pv(t - 2, e_hist.pop(0))
                        e_hist.append(e_ab)
                        # deferred out-projection of the previous outer
                        if pending is not None and t in oj_sched:
                            for j in oj_sched[t]:
                                emit_outproj_chunk(pending[0], pending[1], j)
                        # next batch's projection chunk, spread over the t-loop
                        if hc is not None and t in pp_sched:
                            for part in pp_sched[t]:
                                emit_proj(hc, part)
                    for i, e_h in enumerate(e_hist):
                        emit_pv(nt - len(e_hist) + i, e_h)

                    if dump and o == 0:
                        nc.sync.dma_start(out=qT_d, in_=qT_sb[:])
                        nc.sync.dma_start(out=kT_d, in_=kT_sb[:])
                        nc.sync.dma_start(
                            out=v_d.rearrange("p (t c) -> p t c", c=160),
                            in_=v_sb[:])
                        nc.sync.dma_start(out=e_d, in_=e_hist[-1][:])
                        nc.sync.dma_start(out=x_d, in_=x_tiles[min(4, n_chunks - 1)][:])
                        pv_stage = pvspool.tile([dk + 1, 2 * c.sc], F32,
                                                tag="pvstage")
                        nc.vector.tensor_copy(pv_stage[:], pv_ab[:])
                        nc.sync.dma_start(out=pv_d, in_=pv_stage[:])

                    # free PSUM fast: evict pv data + denom row to SBUF
                    den_sb = rpool.tile([dk + 1, 2 * c.sc], F32, tag="den")
                    pvs = pvspool.tile([dk, 2 * c.sc], F32, tag="pvs")
                    nc.vector.tensor_copy(den_sb[dk:dk + 1, :],
                                          pv_ab[dk:dk + 1, :])
                    nc.vector.tensor_copy(pvs[:], pv_ab[0:dk, :])

                    # normalization chain, all on SBUF, off critical path
                    den0 = rpool.tile([1, 2 * c.sc], F32, tag="den0")
                    rec0 = rpool.tile([1, 2 * c.sc], F32, tag="rec0")
                    # lane-shift the denom row to partition 0 (gpsimd
                    # broadcast and the custom recip both need base 0)
                    nc.sync.dma_start(out=den0[0:1, :],
                                      in_=den_sb[dk:dk + 1, :])
                    with nc.allow_low_precision(
                            reason="softmax denom recip at ~18 bits"):
                        nc.vector.reciprocal_approx_fast(
                            out=rec0[0:1, :], in_=den0[0:1, :])
                    bcs_ab = bcspool.tile([dk, 2 * c.sc], F32, tag="bab")
                    nc.gpsimd.partition_broadcast(
                        bcs_ab[:], rec0[0:1, :], channels=dk)
                    norm128 = npool.tile([128, c.sc], BF16, tag="n128")
                    nrm_b = npool.tile([dk, c.sc], BF16, tag="nbt")
                    nc.vector.tensor_tensor(
                        norm128[0:dk, :], pvs[:, 0:c.sc],
                        bcs_ab[:, 0:c.sc], mybir.AluOpType.mult)
                    nc.vector.tensor_tensor(
                        nrm_b[:], pvs[:, c.sc:2 * c.sc],
                        bcs_ab[:, c.sc:2 * c.sc], mybir.AluOpType.mult)
                    # lane-shift head B into partitions 64..127
                    nc.sync.dma_start(out=norm128[dk:2 * dk, :],
                                      in_=nrm_b[:])

                    pending = (norm128, s0)
                if pending is not None:
                    for j in range(c.sc // 128):
                        emit_outproj_chunk(pending[0], pending[1], j)

    nc.compile()
    return nc


_NC_CACHE = {}


def get_nc(cfg: Cfg | None = None):
    cfg = cfg or Cfg()
    key = (cfg.b, cfg.s, cfg.d, cfg.cpc, cfg.dk)
    if key not in _NC_CACHE:
        _NC_CACHE[key] = _build_nc(cfg)
    return _NC_CACHE[key]


def kernel(x, w_q, b_q, w_k, b_k, w_v, b_v, w_o, b_o,
           a_q, u_q, a_k, u_k, a_v, u_v):
    cfg = Cfg()
    c = cfg
    x = np.asarray(x, np.float32)
    w_q = np.asarray(w_q, np.float32)
    w_k = np.asarray(w_k, np.float32)
    w_v = np.asarray(w_v, np.float32)
    w_o = np.asarray(w_o, np.float32)
    b_q = np.asarray(b_q, np.float32)
    b_k = np.asarray(b_k, np.float32)
    b_v = np.asarray(b_v, np.float32)
    b_o = np.asarray(b_o, np.float32)

    def merge(w, a, u):
        return (w.astype(np.float64)
                + (np.asarray(a, np.float64) @ np.asarray(u, np.float64))
                * SCALING).astype(np.float32)

    wq_eff = merge(w_q, a_q, u_q)
    wk_eff = merge(w_k, a_k, u_k)
    wv_eff = merge(w_v, a_v, u_v)

    import ml_dtypes
    BFNP = ml_dtypes.bfloat16
    xT = np.ascontiguousarray(x.reshape(c.seq, c.d).T.astype(BFNP))
    in_maps = []
    for i in range(N_CORES):
        sl = slice(i * c.cpc, (i + 1) * c.cpc)
        in_maps.append({
            "xT": xT,
            "wq": np.ascontiguousarray(wq_eff[:, sl].astype(BFNP)),
            "wk": np.ascontiguousarray(wk_eff[:, sl].astype(BFNP)),
            "wv": np.ascontiguousarray(wv_eff[:, sl].astype(BFNP)),
            "wo": np.ascontiguousarray(w_o[sl, :].astype(BFNP)),
            "bq": np.ascontiguousarray(b_q[sl]).reshape(c.cpc, 1),
            "bk": np.ascontiguousarray(b_k[sl]).reshape(c.cpc, 1),
        })

    nc = get_nc(cfg)
    res = run_bass_kernel_spmd(nc, in_maps, list(range(N_CORES)))
    out = np.zeros((c.seq, c.d), np.float32)
    for i in range(N_CORES):
        out += res.results[i]["out"]
    # v-bias rides through softmax as a constant row; b_o is plain bias
    out += (b_v @ w_o + b_o).astype(np.float32)
    return out.reshape(B, S, D_MODEL).astype(np.float32)


# revision 11
# speedup vs baseline: 1.1481x; 1.1481x over previous
"""LoRA attention Bass kernel for 8x Trainium2 NeuronCores.

Sharding (Megatron tensor-parallel over heads):
  - Each of the 8 cores owns 2 heads (128 projection columns).
  - q/k/v projections column-sharded; out projection row-sharded;
    per-core partial outputs are summed on the host.
  - LoRA is merged into the base weights on the host (w_eff = w + a@u*scaling),
    exact up to fp32 rounding; host casts x/wq/wk/wv/wo to bf16.

Fused single-loop pipeline (v2). The softmax exp on the scalar engine is
the hard floor (256 ACTIVATE calls x ~1.1us = ~287us/core); everything
else is organized to hide under that pace:
  - No separate projection phase: projection chunk c (512 seq rows of
    q/k/v for batch c//4) is emitted interleaved inside attention outer
    c-4's t-loop, so the PE fills exp-wait gaps with projection matmuls
    while ACT runs flat out. Batch 0 projects in a short prologue.
  - qT/kT computed transposed ([proj_col, seq]) at full PE rate (N=512);
    PSUM evicted by DVE with fused bias-add + bf16 cast (ACT does exp ONLY).
  - v natural layout ([t, d_k] slabs with fused ones columns for the
    softmax denominator) produced by DMA-xbar transposes of the vT
    eviction - zero PE/ACT cost.
  - Attention inner loop per (batch, s-chunk) outer: S^T = K @ Q^T with
    both heads as row-tiled concurrent matmuls (K=64 at tile_position
    (0,0)/(64,0)) into a [128,1024] 2-bank PSUM tile; one exp over
    [128,1024] on ACT into bf16; P@V with lhsT=[v | ones] so the softmax
    denominator falls out of the same matmul (row 64). PV emitted two
    t-steps late so it never heads-of-line blocks on exp.
  - Normalization (DVE/gpsimd, off critical path): denom row lane-shifted
    to partition 0 by a tiny SBUF->SBUF DMA, reciprocal_approx_fast,
    gpsimd partition_broadcast, DVE multiply; head-B half lane-shifted by
    another local DMA. Out-projection of outer i deferred into outer
    i+1's t-loop; PSUM evicted by DVE, DMAed to DRAM.

PSUM budget (8 banks): scores 2x2 + pv 2 + proj 1 + outproj 1.
"""

import numpy as np

import concourse.bass as bass
import concourse.mybir as mybir
import concourse.tile as tile
from concourse import bacc
from concourse.bass_utils import run_bass_kernel_spmd

F32 = mybir.dt.float32
BF16 = mybir.dt.bfloat16
AF = mybir.ActivationFunctionType

N_CORES = 8

# Full-problem dims (hardcoded per spec)
D_MODEL = 1024
N_HEADS = 16
D_K = 64
LORA_R = 8
SCALING = 2.0
B = 4
S = 2048


class Cfg:
    """Kernel build configuration."""

    def __init__(self, b=B, s=S, d=D_MODEL, cpc=128, dk=D_K):
        self.b = b                     # batches
        self.s = s                     # seq per batch
        self.d = d                     # model dim (contraction for projections)
        self.cpc = cpc                 # projection cols per core (2 heads x 64)
        self.dk = dk                   # head dim
        self.seq = b * s               # total rows
        self.nkc = d // 128            # k chunks for projections
        self.sc = 512                  # s-chunk width (free dim of matmuls)
        self.nsc = self.seq // self.sc  # s chunks over the whole input
        self.nt = s // 128             # t chunks per batch
        self.nsb = s // self.sc        # s chunks per batch


def _build_nc(cfg: Cfg, dump: bool = False):
    c = cfg
    nc = bacc.Bacc("TRN2", target_bir_lowering=False, debug=False,
                   num_devices=N_CORES)

    if dump:
        qT_d = nc.dram_tensor("qT_d", [128, c.seq], BF16, kind="ExternalOutput").ap()
        kT_d = nc.dram_tensor("kT_d", [128, c.seq], BF16, kind="ExternalOutput").ap()
        v_d = nc.dram_tensor("v_d", [128, (c.seq // 128) * 160], BF16, kind="ExternalOutput").ap()
        e_d = nc.dram_tensor("e_d", [128, 1024], BF16, kind="ExternalOutput").ap()
        pv_d = nc.dram_tensor("pv_d", [65, 1024], F32, kind="ExternalOutput").ap()
        x_d = nc.dram_tensor("x_d", [128, 8, 512], BF16, kind="ExternalOutput").ap()

    xT = nc.dram_tensor("xT", [c.d, c.seq], BF16, kind="ExternalInput").ap()
    wq = nc.dram_tensor("wq", [c.d, c.cpc], BF16, kind="ExternalInput").ap()
    wk = nc.dram_tensor("wk", [c.d, c.cpc], BF16, kind="ExternalInput").ap()
    wv = nc.dram_tensor("wv", [c.d, c.cpc], BF16, kind="ExternalInput").ap()
    wo = nc.dram_tensor("wo", [c.cpc, c.d], BF16, kind="ExternalInput").ap()
    bq = nc.dram_tensor("bq", [c.cpc, 1], F32, kind="ExternalInput").ap()
    bk = nc.dram_tensor("bk", [c.cpc, 1], F32, kind="ExternalInput").ap()
    out = nc.dram_tensor("out", [c.seq, c.d], F32, kind="ExternalOutput").ap()

    dk = c.dk
    nt = c.nt                     # t-steps per outer (128-row chunks per batch)
    ntc = c.seq // 128            # global 128-row t-chunk count
    n_outers = c.b * c.nsb
    n_chunks = n_outers           # projection chunks, same 512-row granularity
    tpc = c.sc // 128             # t-chunks per projection chunk (4)
    VW = 160                      # v slab width: [0:64]=A, 64=onesA, [80:144]=B, 144=onesB
    HB = 80    # 32-byte-aligned slab offset (DMA-xbar transpose corrupts at 16B-only alignment)

    # per-t-step schedules inside an outer
    oj_sched: dict[int, list[int]] = {}
    for j in range(c.sc // 128):
        oj_sched.setdefault(min(5 + 3 * j, nt - 1), []).append(j)
    # v parts first so the vt eviction + DMA-xbar transposes clear the sync
    # queue early in the outer (the norm lane-shifts queue behind them)
    pp_order = [4, 5, 0, 1, 2, 3]
    pp_sched: dict[int, list[int]] = {}
    for i, p in enumerate(pp_order):
        pp_sched.setdefault(min(2 + 2 * i, nt - 1), []).append(p)

    with tile.TileContext(nc) as tc:
        with tc.tile_pool(name="persist", bufs=1) as persist:
            qT_sb = persist.tile([128, c.seq], BF16, tag="qT")
            kT_sb = persist.tile([128, c.seq], BF16, tag="kT")
            v_sb = persist.tile([128, ntc, VW], BF16, tag="v")
            wq_sb = persist.tile([128, c.nkc, c.cpc], BF16, tag="wq")
            wk_sb = persist.tile([128, c.nkc, c.cpc], BF16, tag="wk")
            wv_sb = persist.tile([128, c.nkc, c.cpc], BF16, tag="wv")
            wo_sb = persist.tile([c.cpc, c.d], BF16, tag="wo")
            bq_sb = persist.tile([c.cpc, 1], F32, tag="bq")
            bk_sb = persist.tile([c.cpc, 1], F32, tag="bk")

            nc.sync.dma_start(out=wq_sb[:], in_=wq.rearrange("(kc p) m -> p kc m", p=128))
            nc.sync.dma_start(out=wk_sb[:], in_=wk.rearrange("(kc p) m -> p kc m", p=128))
            nc.sync.dma_start(out=wv_sb[:], in_=wv.rearrange("(kc p) m -> p kc m", p=128))
            nc.sync.dma_start(out=wo_sb[:], in_=wo[:])
            nc.sync.dma_start(out=bq_sb[:], in_=bq[:])
            nc.sync.dma_start(out=bk_sb[:], in_=bk[:])

            # ones columns for the fused softmax denominator
            ones_f32 = persist.tile([128, 1], F32, tag="ones_f32")
            nc.vector.memset(ones_f32[:], 1.0)
            nc.vector.tensor_copy(
                v_sb[:, :, dk:dk + 1],
                ones_f32[:].unsqueeze(1).to_broadcast([128, ntc, 1]))
            nc.vector.tensor_copy(
                v_sb[:, :, HB + dk:HB + dk + 1],
                ones_f32[:].unsqueeze(1).to_broadcast([128, ntc, 1]))

            bq_bc = bq_sb[:].to_broadcast([c.cpc, c.sc])
            bk_bc = bk_sb[:].to_broadcast([c.cpc, c.sc])

            with tc.tile_pool(name="xin", bufs=5) as xpool, \
                 tc.tile_pool(name="vt", bufs=2) as vtpool, \
                 tc.tile_pool(name="sps", bufs=2, space="PSUM") as spool, \
                 tc.tile_pool(name="pvps", bufs=1, space="PSUM") as pvpool, \
                 tc.tile_pool(name="mps", bufs=1, space="PSUM") as mpool, \
                 tc.tile_pool(name="exp", bufs=6) as epool, \
                 tc.tile_pool(name="norm", bufs=4) as npool, \
                 tc.tile_pool(name="pvs", bufs=2) as pvspool, \
                 tc.tile_pool(name="bcs", bufs=2) as bcspool, \
                 tc.tile_pool(name="rec", bufs=3) as rpool, \
                 tc.tile_pool(name="osb", bufs=3) as osbpool:

                x_tiles: dict[int, bass.AP] = {}
                proj_ps: dict[tuple, bass.AP] = {}

                def issue_x(ci):
                    x_t = xpool.tile([128, c.nkc, c.sc], BF16, tag="x",
                                     name=f"x_{ci}")
                    nc.sync.dma_start(
                        out=x_t[:],
                        in_=xT.rearrange("(kc p) s -> p kc s", p=128)
                        [:, :, ci * c.sc:(ci + 1) * c.sc])
                    x_tiles[ci] = x_t

                def emit_proj(ci, part):
                    """part 0..5 = (q|k|v) x (kc half 0|1) for chunk ci."""
                    which, half = divmod(part, 2)
                    w_sb = (wq_sb, wk_sb, wv_sb)[which]
                    x_t = x_tiles[ci]
                    if half == 0:
                        proj_ps[(ci, which)] = mpool.tile(
                            [128, c.sc], F32, tag="pp",
                            name=f"pp_{ci}_{which}")
                    ps = proj_ps[(ci, which)]
                    h4 = c.nkc // 2
                    for kc in range(half * h4, half * h4 + h4):
                        nc.tensor.matmul(ps[:], w_sb[:, kc, :], x_t[:, kc, :],
                                         start=(kc == 0), stop=(kc == c.nkc - 1))
                    if half == 1:
                        s0c = ci * c.sc
                        if which == 0:
                            nc.vector.tensor_tensor(
                                qT_sb[:, s0c:s0c + c.sc], ps[:], bq_bc,
                                mybir.AluOpType.add)
                        elif which == 1:
                            nc.vector.tensor_tensor(
                                kT_sb[:, s0c:s0c + c.sc], ps[:], bk_bc,
                                mybir.AluOpType.add)
                        else:
                            vt = vtpool.tile([128, c.sc], BF16, tag="vt",
                                             name=f"vt_{ci}")
                            nc.vector.tensor_copy(vt[:], ps[:])
                            tc0 = ci * tpc
                            nc.sync.dma_start_transpose(
                                out=v_sb[:, tc0:tc0 + tpc, 0:dk],
                                in_=vt[0:dk, :])
                            nc.sync.dma_start_transpose(
                                out=v_sb[:, tc0:tc0 + tpc, HB:HB + dk],
                                in_=vt[dk:2 * dk, :])
                        del proj_ps[(ci, which)]

                def emit_outproj_chunk(norm128, s0, j):
                    o_t = osbpool.tile([128, c.d], F32, tag="osb",
                                       name=f"o_t_{s0}_{j}")
                    ew = 512
                    for e in range(c.d // ew):
                        o_ps = mpool.tile([128, ew], F32, tag="o",
                                          name=f"o_ps_{s0}_{j}_{e}")
                        nc.tensor.matmul(
                            o_ps[:],
                            norm128[:, j * 128:(j + 1) * 128],
                            wo_sb[:, e * ew:(e + 1) * ew],
                            start=True, stop=True)
                        nc.vector.tensor_copy(
                            o_t[:, e * ew:(e + 1) * ew], o_ps[:])
                    nc.sync.dma_start(
                        out=out[s0 + j * 128:s0 + (j + 1) * 128, :],
                        in_=o_t[:])

                # ---------------- prologue: batch 0 projections ----------------
                for ci in range(c.nsb + 1):
                    if ci < n_chunks:
                        issue_x(ci)
                for ci in range(c.nsb):
                    for part in range(6):
                        emit_proj(ci, part)

                # ---------------- fused attention + projection loop ----------------
                # out-projection of outer o is deferred TWO outers (consumed in
                # outer o+2) so the normalization chain has a full outer of
                # latency slack and never head-of-line blocks the PE queue.
                pend = []   # FIFO of (norm128, s0)
                for o in range(n_outers):
                    b_i, sb_i = divmod(o, c.nsb)
                    s0 = o * c.sc
                    hc = o + c.nsb if o + c.nsb < n_chunks else None
                    if hc is not None and hc + 1 < n_chunks:
                        issue_x(hc + 1)

                    pv_ab = pvpool.tile([dk + 1, 2 * c.sc], F32, tag="pv")

                    def emit_pv(t, e_ab):
                        tci = b_i * nt + t
                        nc.tensor.matmul(
                            pv_ab[:, 0:c.sc], v_sb[:, tci, 0:dk + 1],
                            e_ab[:, 0:c.sc],
                            start=(t == 0), stop=(t == nt - 1))
                        nc.tensor.matmul(
                            pv_ab[:, c.sc:2 * c.sc],
                            v_sb[:, tci, HB:HB + dk + 1],
                            e_ab[:, c.sc:2 * c.sc],
                            start=(t == 0), stop=(t == nt - 1))

                    e_hist = []
                    for t in range(nt):
                        t0 = b_i * c.s + t * 128
                        s_ab = spool.tile([128, 2 * c.sc], F32, tag="s")
                        nc.tensor.matmul(
                            s_ab[:, 0:c.sc],
                            kT_sb[0:dk, t0:t0 + 128],
                            qT_sb[0:dk, s0:s0 + c.sc],
                            start=True, stop=True,
                            tile_position=(0, 0))
                        nc.tensor.matmul(
                            s_ab[:, c.sc:2 * c.sc],
                            kT_sb[dk:2 * dk, t0:t0 + 128],
                            qT_sb[dk:2 * dk, s0:s0 + c.sc],
                            start=True, stop=True,
                            tile_position=(64, 0))
                        e_ab = epool.tile([128, 2 * c.sc], BF16, tag="e")
                        nc.scalar.activation(e_ab[:], s_ab[:], AF.Exp,
                                             scale=1.0 / np.sqrt(dk))
                        if len(e_hist) == 2:
                            emit_pv(t - 2, e_hist.pop(0))
                        e_hist.append(e_ab)
                        # deferred out-projection from two outers back
                        if len(pend) >= 2 and t in oj_sched:
                            for j in oj_sched[t]:
                                emit_outproj_chunk(pend[0][0], pend[0][1], j)
                        # next batch's projection chunk, spread over the t-loop
                        if hc is not None and t in pp_sched:
                            for part in pp_sched[t]:
                                emit_proj(hc, part)
                    for i, e_h in enumerate(e_hist):
                        emit_pv(nt - len(e_hist) + i, e_h)

                    if dump and o == 0:
                        nc.sync.dma_start(out=qT_d, in_=qT_sb[:])
                        nc.sync.dma_start(out=kT_d, in_=kT_sb[:])
                        nc.sync.dma_start(
                            out=v_d.rearrange("p (t c) -> p t c", c=160),
                            in_=v_sb[:])
                        nc.sync.dma_start(out=e_d, in_=e_hist[-1][:])
                        nc.sync.dma_start(out=x_d, in_=x_tiles[min(4, n_chunks - 1)][:])
                        pv_stage = pvspool.tile([dk + 1, 2 * c.sc], F32,
                                                tag="pvstage")
                        nc.vector.tensor_copy(pv_stage[:], pv_ab[:])
                        nc.sync.dma_start(out=pv_d, in_=pv_stage[:])

                    # free PSUM fast: evict pv data + denom row to SBUF
                    den_sb = rpool.tile([dk + 1, 2 * c.sc], F32, tag="den")
                    pvs = pvspool.tile([dk, 2 * c.sc], F32, tag="pvs")
                    nc.vector.tensor_copy(den_sb[dk:dk + 1, :],
                                          pv_ab[dk:dk + 1, :])
                    nc.vector.tensor_copy(pvs[:], pv_ab[0:dk, :])

                    # normalization chain, all on SBUF, off critical path
                    den0 = rpool.tile([1, 2 * c.sc], F32, tag="den0")
                    rec0 = rpool.tile([1, 2 * c.sc], F32, tag="rec0")
                    # lane-shift the denom row to partition 0 (gpsimd
                    # broadcast and the custom recip both need base 0)
                    nc.sync.dma_start(out=den0[0:1, :],
                                      in_=den_sb[dk:dk + 1, :])
                    with nc.allow_low_precision(
                            reason="softmax denom recip at ~18 bits"):
                        nc.vector.reciprocal_approx_fast(
                            out=rec0[0:1, :], in_=den0[0:1, :])
                    bcs_ab = bcspool.tile([dk, 2 * c.sc], F32, tag="bab")
                    nc.gpsimd.partition_broadcast(
                        bcs_ab[:], rec0[0:1, :], channels=dk)
                    norm128 = npool.tile([128, c.sc], BF16, tag="n128")
                    nrm_b = npool.tile([dk, c.sc], BF16, tag="nbt")
                    nc.vector.tensor_tensor(
                        norm128[0:dk, :], pvs[:, 0:c.sc],
                        bcs_ab[:, 0:c.sc], mybir.AluOpType.mult)
                    nc.vector.tensor_tensor(
                        nrm_b[:], pvs[:, c.sc:2 * c.sc],
                        bcs_ab[:, c.sc:2 * c.sc], mybir.AluOpType.mult)
                    # lane-shift head B into partitions 64..127
                    nc.sync.dma_start(out=norm128[dk:2 * dk, :],
                                      in_=nrm_b[:])

                    if len(pend) >= 2:
                        pend.pop(0)
                    pend.append((norm128, s0))
                for norm128_f, s0_f in pend:
                    for j in range(c.sc // 128):
                        emit_outproj_chunk(norm128_f, s0_f, j)

    nc.compile()
    return nc


_NC_CACHE = {}


def get_nc(cfg: Cfg | None = None):
    cfg = cfg or Cfg()
    key = (cfg.b, cfg.s, cfg.d, cfg.cpc, cfg.dk)
    if key not in _NC_CACHE:
        _NC_CACHE[key] = _build_nc(cfg)
    return _NC_CACHE[key]


def kernel(x, w_q, b_q, w_k, b_k, w_v, b_v, w_o, b_o,
           a_q, u_q, a_k, u_k, a_v, u_v):
    cfg = Cfg()
    c = cfg
    x = np.asarray(x, np.float32)
    w_q = np.asarray(w_q, np.float32)
    w_k = np.asarray(w_k, np.float32)
    w_v = np.asarray(w_v, np.float32)
    w_o = np.asarray(w_o, np.float32)
    b_q = np.asarray(b_q, np.float32)
    b_k = np.asarray(b_k, np.float32)
    b_v = np.asarray(b_v, np.float32)
    b_o = np.asarray(b_o, np.float32)

    def merge(w, a, u):
        return (w.astype(np.float64)
                + (np.asarray(a, np.float64) @ np.asarray(u, np.float64))
                * SCALING).astype(np.float32)

    wq_eff = merge(w_q, a_q, u_q)
    wk_eff = merge(w_k, a_k, u_k)
    wv_eff = merge(w_v, a_v, u_v)

    import ml_dtypes
    BFNP = ml_dtypes.bfloat16
    xT = np.ascontiguousarray(x.reshape(c.seq, c.d).T.astype(BFNP))
    in_maps = []
    for i in range(N_CORES):
        sl = slice(i * c.cpc, (i + 1) * c.cpc)
        in_maps.append({
            "xT": xT,
            "wq": np.ascontiguousarray(wq_eff[:, sl].astype(BFNP)),
            "wk": np.ascontiguousarray(wk_eff[:, sl].astype(BFNP)),
            "wv": np.ascontiguousarray(wv_eff[:, sl].astype(BFNP)),
            "wo": np.ascontiguousarray(w_o[sl, :].astype(BFNP)),
            "bq": np.ascontiguousarray(b_q[sl]).reshape(c.cpc, 1),
            "bk": np.ascontiguousarray(b_k[sl]).reshape(c.cpc, 1),
        })

    nc = get_nc(cfg)
    res = run_bass_kernel_spmd(nc, in_maps, list(range(N_CORES)))
    out = np.zeros((c.seq, c.d), np.float32)
    for i in range(N_CORES):
        out += res.results[i]["out"]
    # v-bias rides through softmax as a constant row; b_o is plain bias
    out += (b_v @ w_o + b_o).astype(np.float32)
    return out.reshape(B, S, D_MODEL).astype(np.float32)
